# revision 29
# baseline (speedup 1.0000x reference)
"""Trainium2 Bass kernel for the categorical-loss nn.Module.

Computation (matching the single-device jax reference):
    gens    = argmax(logits, axis=-1)                     # [B,T]
    sel     = assoc_mask[gens]                            # [B,T,C]
    attnsum = einsum('btc,bct->bt', sel, attns)
    attnloss = mean(where(any(sel,-1), (1-attnsum)^2, 0))
    nll     = logsumexp(logits) - logits[target]
    xent    = sum((target!=0)*nll) / sum(target!=0)
    out     = xent + attnloss                             # f32 scalar

Sharding: data-parallel on the flattened (B*T)=4096 rows, 512 rows per
core across 8 cores; assoc_mask replicated.  The logits stream is cast
to bf16 on the host (tolerance is 2e-2; measured end error ~2e-6), so
each core streams 32 MB instead of 64 MB.  Per [128,~8192] bf16 chunk:
a 5-level tensor_tensor max fold tree (2x DVE perf mode; groups of 256)
plus one tiny 8->1 tensor_reduce gives the per-group maxes, and one
ScalarE Exp pass with accum_out gives the free-dim sum of exp.  The
exact (bf16) first-occurrence argmax is recovered from the 125 group
maxes per row (max + max_index), an indirect re-gather of the winning
256-element group from DRAM, and a second max/max_index.  The assoc
row and target logit are indirect-gathered on device.  Each row
group's resolution chain is interleaved between the next row group's
chunk ops so the in-order engines never stall on the gather round
trips.  Per-core output is a tiny [128,25] partial tensor ([17 exp-sum
| 4 target-logit | 4 attn-term] columns) plus the [128,4] argmax
indices; the host does the final log + scalar reduction.
"""

import ml_dtypes
import numpy as np

import concourse.bass as bass
from concourse import bacc, mybir
from concourse.bass_utils import run_bass_kernel_spmd
from concourse.tile import TileContext

# Problem shape (hardcoded; kernel.py must be self-contained).
B, T, V, C = 4, 1024, 32000, 64
NCORES = 8
P = 128                    # SBUF partitions
R = (B * T) // NCORES      # rows (positions) per core = 512
RG = R // P                # row-groups per core = 4
S = 256                    # group size for two-level argmax
G = V // S                 # groups per row = 125
CHMAX = 8192               # max chunk free size

# per-row-group chunk column spans (multiples of S).  rg0 is split into
# geometrically growing chunks so the ScalarE exp stream starts early and
# never starves while the DMA queues ramp up.
_RG0_SIZES = [2048, 2560, 3072, 4096, 5120, 6144, 6656, 2304]
_rg0 = []
_lo = 0
for _w in _RG0_SIZES:
    _rg0.append((_lo, _lo + _w))
    _lo += _w
assert _lo == V
CHUNK_SPANS = [
    _rg0,
    [(0, 8192), (8192, 16384), (16384, 24576), (24576, 32000)],
    [(0, 8192), (8192, 16384), (16384, 24576), (24576, 32000)],
    [(0, 8192), (8192, 16384), (16384, 24576), (24576, 32000)],
]
SS_OFF = [0]
for _sp in CHUNK_SPANS:
    SS_OFF.append(SS_OFF[-1] + len(_sp))
NSS = SS_OFF[-1]  # 20 exp-sum columns

# GpSimd Schraudolph slices: (rg, k, act_hi); GpSimd computes bit-cast
# exp int32(A*x+B) for columns [act_hi, w) of that chunk and a late DVE
# reduce sums the f32 reinterpretation; the host divides the resulting
# column by CAL (the analytic E[(1+m)/2^m] inflation of the bit trick).
PIECES = [(1, 0, 4096), (2, 0, 4096)]
NPS = len(PIECES)
CAL = 1.0407590
SCHRA_A = 12102203.161561485   # 2^23 / ln 2
SCHRA_B = float(127 * 2 ** 23)

# output column layout: [ssum (NSS) | pool ssum (NPS) | tv (RG) | attn (RG)]
OUT_SS = 0
OUT_PS = NSS
OUT_TV = NSS + NPS
OUT_AT = OUT_TV + RG
OUT_W = OUT_AT + RG

# fold-tree levels: 256 -> 128 -> 64 -> 32 -> 16 -> 8, then reduce 8 -> 1
FOLD_H = [128, 64, 32, 16, 8]
SCR_W = sum(FOLD_H)  # 248 scratch columns per group

_DT = mybir.dt


def build_nc() -> bass.Bass:
    """Build the per-core Bass program (SPMD: identical on all cores)."""
    nc = bacc.Bacc(
        "TRN2", target_bir_lowering=False, debug=False, num_devices=NCORES
    )

    lg = nc.dram_tensor("lg", [R * V], _DT.bfloat16, kind="ExternalInput")
    tofs = nc.dram_tensor("tofs", [P, RG], _DT.uint32, kind="ExternalInput")
    attn_t = nc.dram_tensor("attn_t", [R, C], _DT.float32, kind="ExternalInput")
    amask = nc.dram_tensor("amask", [V, C], _DT.float32, kind="ExternalInput")
    out = nc.dram_tensor("out", [P, OUT_W], _DT.float32, kind="ExternalOutput")
    gens_out = nc.dram_tensor("gens", [P, RG], _DT.uint32, kind="ExternalOutput")

    # Views of the logits shard.
    lg2d = lg[:].rearrange("(r v) -> r v", v=V)      # [512, 32000]
    lg_s = lg[:].rearrange("(n s) -> n s", s=S)      # [512*125, 256]
    lg_e = lg[:].rearrange("(n o) -> n o", o=1)      # [512*32000, 1]

    fp32 = _DT.float32
    bf16 = _DT.bfloat16
    u32 = _DT.uint32
    AX = mybir.AxisListType.X
    OP = mybir.AluOpType

    with TileContext(nc) as tc:
        with (
            tc.tile_pool(name="chunks", bufs=6) as chunks,
            tc.tile_pool(name="scr", bufs=3) as scrp,
            tc.tile_pool(name="expo", bufs=1) as expo,
            tc.tile_pool(name="ti", bufs=2) as tip,
            tc.tile_pool(name="small", bufs=2) as small,
            tc.tile_pool(name="consts", bufs=1) as consts,
        ):
            # ---- preamble: constants + everything independent of logits ----
            rowbase_i = consts.tile([P, RG], _DT.int32)
            nc.gpsimd.iota(
                rowbase_i[:], [[G * P, RG]], base=0, channel_multiplier=G
            )
            rowbase_f = consts.tile([P, RG], fp32)
            nc.vector.tensor_copy(out=rowbase_f[:], in_=rowbase_i[:])

            # preamble loads go on the scalar HWDGE ring so chunk(0,0) is
            # the first transfer on the sync ring
            tofs_sb = consts.tile([P, RG], u32)
            nc.scalar.dma_start(out=tofs_sb[:], in_=tofs[:])

            out_sb = consts.tile([P, OUT_W], fp32)
            gens_sb = consts.tile([P, RG], u32)

            # attns, transposed on host to [512, 64]: load as [p, rg, c]
            at_all = consts.tile([P, RG, C], fp32)
            nc.scalar.dma_start(
                out=at_all[:],
                in_=attn_t[:].rearrange("(g p) c -> p g c", p=P),
            )

            # target-logit gathers: independent of everything downstream.
            # The stream is bf16, so gather into a bf16 staging tile and
            # convert into the f32 out columns at the tail.
            tv_bf = consts.tile([P, RG], bf16)
            for rg in range(RG):
                nc.gpsimd.indirect_dma_start(
                    out=tv_bf[:, rg:rg + 1],
                    out_offset=None,
                    in_=lg_e,
                    in_offset=bass.IndirectOffsetOnAxis(
                        ap=tofs_sb[:, rg:rg + 1], axis=0
                    ),
                )

            # ---- streaming + interleaved resolution ----
            mc_tiles = {}
            st = {}  # per-rg resolution state (small tiles)

            from concourse.tile import add_dep_helper

            red = {}  # (rg, k) -> last max instruction, for ordering edges

            def after(binst, dep, why):
                # Ordering-only edge: binst must not be scheduled before dep.
                add_dep_helper(binst.ins, dep.ins, sync=False, reason=why)

            tiles = {}
            ti_tiles = {}
            piece_hi = {(rg, k): hi for rg, k, hi in PIECES}

            def chunk_dma(rg, k):
                lo, hi = CHUNK_SPANS[rg][k]
                w = hi - lo
                t = chunks.tile([P, CHMAX], bf16, name=f"t_{rg}_{k}", tag="t")
                tiles[(rg, k)] = t
                # alternate the two HWDGE rings (sync / scalar sequencers)
                dma_eng = nc.sync if (SS_OFF[rg] + k) % 2 == 0 else nc.scalar
                dma_eng.dma_start(
                    out=t[:, :w],
                    in_=lg2d[rg * P:(rg + 1) * P, lo:hi],
                )

            def chunk_compute(rg, k):
                lo, hi = CHUNK_SPANS[rg][k]
                w = hi - lo
                gpc = w // S
                t = tiles[(rg, k)]
                # grouped max: 5-level fold tree (2x DVE mode on bf16) then
                # one tiny 8->1 reduce into the group-max table
                v = t[:, :w].rearrange("p (g s) -> p g s", s=S)
                scr = scrp.tile(
                    [P, gpc, SCR_W], bf16, name=f"s_{rg}_{k}", tag="s"
                )
                prev = v
                off = 0
                for h in FOLD_H:
                    cur = scr[:, :, off:off + h]
                    nc.vector.tensor_tensor(
                        out=cur,
                        in0=prev[:, :, 0:h],
                        in1=prev[:, :, h:2 * h],
                        op=OP.max,
                    )
                    prev = cur
                    off += h
                red[(rg, k)] = nc.vector.tensor_reduce(
                    out=mc_tiles[rg][:, lo // S:hi // S],
                    in_=prev,
                    axis=AX,
                    op=OP.max,
                )
                act_hi = piece_hi.get((rg, k), w)
                sscol = OUT_SS + SS_OFF[rg] + k
                eo = expo.tile([P, CHMAX], bf16, name=f"eo_{rg}_{k}", tag="eo")
                nc.scalar.activation(
                    out=eo[:, :act_hi],
                    in_=t[:, :act_hi],
                    func=mybir.ActivationFunctionType.Exp,
                    accum_out=out_sb[:, sscol:sscol + 1],
                )

            def pool_piece(idx):
                rg, k, act_hi = PIECES[idx]
                lo, hi = CHUNK_SPANS[rg][k]
                pw = hi - lo - act_hi
                t = tiles[(rg, k)]
                ti = tip.tile([P, 4096], _DT.int32, name=f"ti_{idx}", tag="ti")
                nc.gpsimd.tensor_scalar(
                    out=ti[:, :pw], in0=t[:, act_hi:act_hi + pw],
                    scalar1=SCHRA_A, scalar2=SCHRA_B,
                    op0=OP.mult, op1=OP.add,
                )
                ti_tiles[idx] = (ti, pw)

            def pool_reduce(idx):
                ti, pw = ti_tiles[idx]
                nc.vector.tensor_reduce(
                    out=out_sb[:, OUT_PS + idx:OUT_PS + idx + 1],
                    in_=ti[:, :pw].bitcast(fp32), axis=AX, op=OP.add,
                )

            def part1(rg, dep=None):
                # global max + winning group; issue the group re-gather
                mc = mc_tiles[rg]
                m8 = small.tile([P, 8], bf16, name=f"m8_{rg}", tag="m8")
                i = nc.vector.max(out=m8[:], in_=mc[:])
                if dep is not None:
                    after(i, dep, f"part1({rg}) placement")
                g8 = small.tile([P, 8], u32, name=f"g8_{rg}", tag="g8")
                nc.vector.max_index(g8[:], m8[:], mc[:])
                g8f = small.tile([P, 1], fp32, name=f"g8f_{rg}", tag="g8f")
                nc.vector.tensor_copy(out=g8f[:], in_=g8[:, 0:1])
                gidxf = small.tile([P, 1], fp32, name=f"gxf_{rg}", tag="gxf")
                nc.vector.tensor_tensor(
                    out=gidxf[:], in0=rowbase_f[:, rg:rg + 1], in1=g8f[:],
                    op=OP.add,
                )
                gidx = small.tile([P, 1], u32, name=f"gx_{rg}", tag="gx")
                nc.vector.tensor_copy(out=gidx[:], in_=gidxf[:])
                grp = small.tile([P, S], bf16, name=f"grp_{rg}", tag="grp")
                nc.gpsimd.indirect_dma_start(
                    out=grp[:],
                    out_offset=None,
                    in_=lg_s,
                    in_offset=bass.IndirectOffsetOnAxis(ap=gidx[:, :1], axis=0),
                )
                st[rg] = {"g8f": g8f, "grp": grp}

            def part2(rg, dep=None):
                # index within the winning group -> gens; issue assoc gather
                grp = st[rg]["grp"]
                mg8 = small.tile([P, 8], bf16, name=f"mg8_{rg}", tag="mg8")
                i = nc.vector.max(out=mg8[:], in_=grp[:])
                if dep is not None:
                    after(i, dep, f"part2({rg}) placement")
                j8 = small.tile([P, 8], u32, name=f"j8_{rg}", tag="j8")
                nc.vector.max_index(j8[:], mg8[:], grp[:])
                j8f = small.tile([P, 1], fp32, name=f"j8f_{rg}", tag="j8f")
                nc.vector.tensor_copy(out=j8f[:], in_=j8[:, 0:1])
                gensf = small.tile([P, 1], fp32, name=f"gf_{rg}", tag="gf")
                nc.vector.tensor_scalar(
                    out=gensf[:], in0=st[rg]["g8f"][:], scalar1=float(S),
                    scalar2=None, op0=OP.mult,
                )
                nc.vector.tensor_tensor(
                    out=gensf[:], in0=gensf[:], in1=j8f[:], op=OP.add
                )
                nc.vector.tensor_copy(out=gens_sb[:, rg:rg + 1], in_=gensf[:])
                sel = small.tile([P, C], fp32, name=f"sel_{rg}", tag="sel")
                nc.gpsimd.indirect_dma_start(
                    out=sel[:],
                    out_offset=None,
                    in_=amask[:],
                    in_offset=bass.IndirectOffsetOnAxis(
                        ap=gens_sb[:, rg:rg + 1], axis=0
                    ),
                )
                st[rg]["sel"] = sel

            def part3(rg, dep=None):
                # attn loss term
                sel = st[rg]["sel"]
                has = small.tile([P, 1], fp32, name=f"has_{rg}", tag="has")
                i = nc.vector.tensor_reduce(
                    out=has[:], in_=sel[:], axis=AX, op=OP.max
                )
                if dep is not None:
                    after(i, dep, f"part3({rg}) placement")
                nc.vector.tensor_tensor(
                    out=sel[:], in0=sel[:], in1=at_all[:, rg, :], op=OP.mult
                )
                asum = small.tile([P, 1], fp32, name=f"as_{rg}", tag="as")
                nc.vector.tensor_reduce(out=asum[:], in_=sel[:], axis=AX, op=OP.add)
                u1 = small.tile([P, 1], fp32, name=f"u1_{rg}", tag="u1")
                nc.vector.tensor_scalar(
                    out=u1[:], in0=asum[:], scalar1=-1.0, scalar2=1.0,
                    op0=OP.mult, op1=OP.add,
                )
                nc.vector.tensor_tensor(out=u1[:], in0=u1[:], in1=u1[:], op=OP.mult)
                nc.vector.tensor_tensor(
                    out=out_sb[:, OUT_AT + rg:OUT_AT + rg + 1],
                    in0=u1[:], in1=has[:], op=OP.mult,
                )

            # Schedule: each rg's resolution chain is stretched across the
            # next two row-group windows so the indirect-gather round trips
            # (~11-16us under full streaming load) hide behind big folds.
            def alloc_mc(rg):
                mc_tiles[rg] = small.tile(
                    [P, G], bf16, name=f"mc_{rg}", tag="mc"
                )

            # flat chunk order; DMA configs emitted one chunk ahead of the
            # activations so scalar-ring configs aren't convoyed behind
            # activation semaphore waits on the shared sequencer
            order = [(rg, k) for rg in range(RG)
                     for k in range(len(CHUNK_SPANS[rg]))]
            emitted = [False] * len(order)

            def emit(i):
                if i + 1 < len(order) and not emitted[i + 1]:
                    chunk_dma(*order[i + 1])
                    emitted[i + 1] = True
                chunk_compute(*order[i])

            alloc_mc(0)
            chunk_dma(*order[0])
            emitted[0] = True
            idx = 0
            for k in range(len(CHUNK_SPANS[0])):
                emit(idx)
                idx += 1
            for rg in range(1, RG):
                alloc_mc(rg)
                emit(idx)          # chunk(rg, 0)
                idx += 1
                if rg == 1:
                    pool_piece(0)
                if rg == 2:
                    pool_piece(1)
                    pool_reduce(0)
                if rg == 3:
                    pool_reduce(1)
                if rg >= 2:
                    part3(rg - 2, dep=red[(rg, 0)])
                part1(rg - 1, dep=red[(rg, 0)])
                emit(idx)          # chunk(rg, 1)
                idx += 1
                emit(idx)          # chunk(rg, 2)
                idx += 1
                if rg < RG - 1:
                    part2(rg - 1, dep=red[(rg, 2)])
                emit(idx)          # chunk(rg, 3)
                idx += 1
            # Tail: the last row-group's chain is latency-critical; issue its
            # grp gather FIRST on the (serialized) SWDGE queue, then the
            # leftover rg2 parts and the final sel gathers.
            part1(RG - 1)
            part2(RG - 2, dep=red[(RG - 1, 3)])
            part2(RG - 1)
            part3(RG - 2)
            part3(RG - 1)

            # convert the bf16 target-logit gathers into the f32 out columns
            nc.vector.tensor_copy(
                out=out_sb[:, OUT_TV:OUT_TV + RG], in_=tv_bf[:]
            )

            # ship everything that doesn't depend on rg3's chain as soon as
            # it's ready; only the last columns wait for the tail
            nc.sync.dma_start(
                out=out[:, 0:OUT_AT + RG - 1], in_=out_sb[:, 0:OUT_AT + RG - 1]
            )
            nc.sync.dma_start(
                out=out[:, OUT_AT + RG - 1:OUT_W],
                in_=out_sb[:, OUT_AT + RG - 1:OUT_W],
            )
            nc.sync.dma_start(out=gens_out[:], in_=gens_sb[:])

    nc.compile()
    return nc


_NC_CACHE: list = []


def _get_nc() -> bass.Bass:
    if not _NC_CACHE:
        _NC_CACHE.append(build_nc())
    return _NC_CACHE[0]


def make_in_maps(logits, targets, attns, assoc_mask):
    """Host-side sharding: per-core input dicts."""
    logits = np.asarray(logits, dtype=np.float32)
    targets = np.asarray(targets).astype(np.int64)
    attns = np.asarray(attns, dtype=np.float32)
    amask_f = np.ascontiguousarray(np.asarray(assoc_mask).astype(np.float32))

    lg_all = logits.reshape(B * T, V)
    tflat = targets.reshape(B * T)

    in_maps = []
    for c in range(NCORES):
        r0 = c * R
        lg_c = np.ascontiguousarray(
            lg_all[r0:r0 + R].astype(ml_dtypes.bfloat16)
        ).reshape(R * V)
        tgt_c = tflat[r0:r0 + R]
        # flat element offset of the target logit within this core's shard,
        # laid out [partition, row-group]: row r = rg*128 + p
        tofs_c = (np.arange(R, dtype=np.int64) * V + tgt_c).reshape(RG, P).T
        b = r0 // T
        t0 = r0 % T
        attn_c = np.ascontiguousarray(attns[b, :, t0:t0 + R].T)  # [512, 64]
        in_maps.append({
            "lg": lg_c,
            "tofs": np.ascontiguousarray(tofs_c).astype(np.uint32),
            "attn_t": attn_c,
            "amask": amask_f,
        })
    return in_maps


def combine_results(results, targets):
    """Host-side reduction of the per-core [128, OUT_W] partials."""
    targets = np.asarray(targets).astype(np.int64)
    tflat = targets.reshape(B * T)
    wnll = 0.0
    wsum = 0.0
    asq = 0.0
    piece_rg = [pc[0] for pc in PIECES]
    for c in range(NCORES):
        o = np.asarray(results[c]["out"], dtype=np.float64)  # [128, OUT_W]
        ssum = np.stack(
            [
                o[:, OUT_SS + SS_OFF[rg]:OUT_SS + SS_OFF[rg + 1]].sum(axis=1)
                for rg in range(RG)
            ],
            axis=1,
        )
        for i, rg in enumerate(piece_rg):
            ssum[:, rg] += o[:, OUT_PS + i] / CAL
        lse = np.log(ssum)                     # [128, RG]
        tv = o[:, OUT_TV:OUT_TV + RG]
        nll = (lse - tv).T.reshape(R)          # row r = rg*128 + p
        attn_term = o[:, OUT_AT:OUT_AT + RG].T.reshape(R)
        tgt_c = tflat[c * R:(c + 1) * R]
        w = (tgt_c != 0).astype(np.float64)
        wnll += float((w * nll).sum())
        wsum += float(w.sum())
        asq += float(attn_term.sum())
    loss = wnll / wsum + asq / float(B * T)
    return np.array(loss, dtype=np.float32)


def kernel(**inputs) -> np.ndarray:
    in_maps = make_in_maps(
        inputs["logits"], inputs["targets"], inputs["attns"],
        inputs["assoc_mask"],
    )
    nc = _get_nc()
    res = run_bass_kernel_spmd(nc, in_maps, core_ids=list(range(NCORES))).results
    return combine_results(res, inputs["targets"])
